# revision 13
# baseline (speedup 1.0000x reference)
"""MoE (top-2 of 8 experts + shared SwiGLU) Trainium2 kernel.

Strategy: data-parallel over tokens across 8 NeuronCores (1024 tokens each).
Each core runs an identical program:
  - gate softmax + top-2 on its token slice (fp32r matmuls + DVE top-8)
  - shared-expert SwiGLU over the slice, written to the output first
  - on-device compaction: per-expert gathered token index lists via a
    triangular-matmul prefix sum + indirect-DMA scatter
  - per expert: indirect gather of x rows -> PE transpose -> SwiGLU (fp32r)
    -> scale by routing weight -> indirect scatter-ADD into the output slice
Output per core is its own [1024, 2048] slice; the host just concatenates.
"""

import math
from contextlib import ExitStack
from functools import lru_cache

import numpy as np

import concourse.bass as bass
import concourse.mybir as mybir
import concourse.tile as tile
from concourse import bacc
from concourse.bass_utils import run_bass_kernel_spmd
from concourse.masks import make_identity

F32 = mybir.dt.float32
F32R = mybir.dt.float32r
I32 = mybir.dt.int32
AF = mybir.ActivationFunctionType
OP = mybir.AluOpType

P = 128

# Full-problem dims (graded input is B=4,S=2048,D=2048,E=8,I=1408,SI=2816)
FULL = dict(TS=1024, D=2048, E=8, I=1408, SI=2816, C=384)
N_CORES = 8
BIG = 1.0e9  # pad sentinel added to scatter offsets (forces bounds-check skip)


def build_moe(nc, tc, ctx, io, dims):
    """Emit the tile program. io: dict of DRAM APs. dims: dict of sizes."""
    TS, D, E, I, SI, C = (dims[k] for k in ("TS", "D", "E", "I", "SI", "C"))
    NT = TS // P          # token tiles in slice
    ND = D // P           # d (model dim) tiles
    NI = I // P           # routed inter-dim tiles
    NSI = SI // P         # shared inter-dim tiles
    NCT = C // P          # capacity tiles per expert
    NH = 2 if NT % 2 == 0 else 1   # token halves for gate+shared phases
    TSH = TS // NH        # tokens per half
    NTH = TSH // P
    DCH = min(512, D)     # moving chunk over d (mm2 outputs)
    N_DCH = D // DCH
    SIBLK = 2             # shared si-tiles per weight staging block
    EC = E * C
    assert EC % P == 0

    xs, xT, gwT = io["xs"], io["xT"], io["gwT"]
    w1T, w3T, w2T = io["w1T"], io["w3T"], io["w2T"]
    sw1T, sw3T, sw2T = io["sw1T"], io["sw3T"], io["sw2T"]
    ltri, iota8, ecols = io["ltri"], io["iota8"], io["ecols"]
    out = io["out"]
    idx_dram = io["idx_dram"]
    s_dram = io["s_dram"]

    const_pool = ctx.enter_context(tc.tile_pool(name="const", bufs=1))

    identity = const_pool.tile([P, P], F32)
    make_identity(nc, identity[:])
    ltri_sb = const_pool.tile([P, P], F32R)
    nc.sync.dma_start(out=ltri_sb[:], in_=ltri[:].bitcast(F32R))
    iota8_sb = const_pool.tile([P, 8], I32)
    nc.sync.dma_start(out=iota8_sb[:], in_=iota8[:])
    ecols_sb = const_pool.tile([1, NT * E], F32)
    nc.sync.dma_start(out=ecols_sb[:], in_=ecols[:])
    ones_f = const_pool.tile([P, 1], F32)
    nc.vector.memset(ones_f[:], 1.0)
    ones_col = const_pool.tile([P, 1], F32R)
    nc.vector.tensor_copy(ones_col[:], ones_f[:].bitcast(F32R))
    ones_rf = const_pool.tile([1, P], F32)
    nc.vector.memset(ones_rf[:], 1.0)
    ones_row = const_pool.tile([1, P], F32R)
    nc.vector.tensor_copy(ones_row[:], ones_rf[:].bitcast(F32R))
    # gate weights in TRUE fp32: top-2 selection must match the fp32 reference,
    # fp32r's ~1e-4 noise flips near-tied expert choices.
    gwT_sb = []
    for d in range(ND):
        t = const_pool.tile([P, E], F32, name=f"gwT_{d}", tag=f"gwT_{d}")
        nc.sync.dma_start(out=t[:], in_=gwT[d * P:(d + 1) * P, :])
        gwT_sb.append(t)

    # init idx_dram to TS (pad sentinel -> gathers x zero-row, scatters skipped)
    with tc.tile_pool(name="init", bufs=1) as initp:
        it = initp.tile([P, EC // P], I32)
        nc.vector.memset(it[:], TS)
        nc.sync.dma_start(out=idx_dram[:].rearrange("(a b) o -> a (b o)", a=P),
                          in_=it[:])
        st = initp.tile([1, E], F32)
        nc.vector.memset(st[:], 0.0)
        nc.sync.dma_start(
            out=s_dram[:].rearrange("(e t) o -> t (e o)", e=E)[TS:TS + 1, :],
            in_=st[:])

    # ============ Phase 1+2+3 per token half: gate, shared mm1, shared mm2 =====
    rt_pool = ctx.enter_context(tc.tile_pool(name="routing", bufs=1))
    m_all = rt_pool.tile([P, NT * E], F32R)  # top-2 masks, col = j*E + e

    for h in range(NH):
        with tc.tile_pool(name="xt", bufs=1) as xt_pool:
            xT_sb = []
            for d in range(ND):
                t = xt_pool.tile([P, TSH], F32R, name=f"xT_{d}", tag=f"xT_{d}")
                nc.sync.dma_start(
                    out=t[:],
                    in_=xT[d * P:(d + 1) * P, h * TSH:(h + 1) * TSH].bitcast(F32R))
                xT_sb.append(t)

            # ---- gate ----
            with tc.tile_pool(name="gate_sb", bufs=2) as gsb, \
                 tc.tile_pool(name="gate_ps", bufs=2, space="PSUM") as gps:
                for jj in range(NTH):
                    j = h * NTH + jj
                    sc_ps = gps.tile([P, E], F32, space="PSUM", name="sc")
                    for d in range(ND):
                        # fp32 x tile for the gate (exact top-2 selection)
                        xf = gsb.tile([P, P], F32, name="xf")
                        nc.sync.dma_start(
                            out=xf[:],
                            in_=xT[d * P:(d + 1) * P,
                                   h * TSH + jj * P:h * TSH + (jj + 1) * P])
                        nc.tensor.matmul(
                            out=sc_ps[:],
                            lhsT=xf[:],
                            rhs=gwT_sb[d][:],
                            start=(d == 0), stop=(d == ND - 1),
                        )
                    es = gsb.tile([P, E], F32, name="es")
                    nc.scalar.activation(es[:], sc_ps[:], AF.Exp)
                    zsum = gsb.tile([P, 1], F32, name="zsum")
                    nc.vector.tensor_reduce(zsum[:], es[:],
                                            axis=mybir.AxisListType.X, op=OP.add)
                    rec = gsb.tile([P, 1], F32, name="rec")
                    nc.vector.reciprocal(rec[:], zsum[:])
                    prob = gsb.tile([P, E], F32, name="prob")
                    nc.vector.tensor_scalar_mul(prob[:], es[:], rec[:, :1])
                    top8 = gsb.tile([P, 8], F32, name="top8")
                    nc.vector.max(out=top8[:], in_=prob[:])
                    # mask = prob >= second_max  (top-2)
                    nc.vector.tensor_tensor(
                        out=m_all[:, j * E:(j + 1) * E],
                        in0=prob[:], in1=top8[:, 1:2].to_broadcast([P, E]),
                        op=OP.is_ge,
                    )
                    # routing weight s = prob * mask -> s_dram[e, j*P+p]
                    sval = gsb.tile([P, E], F32, name="sval")
                    nc.vector.tensor_tensor(
                        out=sval[:], in0=prob[:],
                        in1=m_all[:, j * E:(j + 1) * E].bitcast(F32), op=OP.mult)
                    nc.sync.dma_start(
                        out=s_dram[:].rearrange(
                            "(e t) o -> t (e o)", e=E)[j * P:(j + 1) * P, :],
                        in_=sval[:],
                    )

            # ---- shared mm1: gS = silu(sw1 x) * (sw3 x) over this half ----
            gs_tiles = []
            with tc.tile_pool(name="gs", bufs=1) as gs_pool:
                for si in range(NSI):
                    gs_tiles.append(
                        gs_pool.tile([P, TSH], F32R, name=f"gs_{si}", tag=f"gs_{si}"))
                n_siblk = math.ceil(NSI / SIBLK)
                with tc.tile_pool(name="sh1_w", bufs=2) as swp, \
                     tc.tile_pool(name="sh1_sb", bufs=3) as ssb, \
                     tc.tile_pool(name="sh1_ps", bufs=2, space="PSUM") as sps:
                    for blk in range(n_siblk):
                        si0 = blk * SIBLK
                        nsi = min(SIBLK, NSI - si0)
                        w1b = swp.tile([P, ND, SIBLK * P], F32R, name="sw1b", tag="sw1b")
                        w3b = swp.tile([P, ND, SIBLK * P], F32R, name="sw3b", tag="sw3b")
                        for d in range(ND):
                            nc.sync.dma_start(
                                out=w1b[:, d, :nsi * P],
                                in_=sw1T[d * P:(d + 1) * P,
                                         si0 * P:(si0 + nsi) * P].bitcast(F32R))
                            nc.sync.dma_start(
                                out=w3b[:, d, :nsi * P],
                                in_=sw3T[d * P:(d + 1) * P,
                                         si0 * P:(si0 + nsi) * P].bitcast(F32R))
                        for q in range(nsi):
                            si = si0 + q
                            h1 = sps.tile([P, TSH], F32, space="PSUM", name="h1")
                            h3 = sps.tile([P, TSH], F32, space="PSUM", name="h3")
                            for d in range(ND):
                                nc.tensor.matmul(
                                    out=h1[:], lhsT=w1b[:, d, q * P:(q + 1) * P],
                                    rhs=xT_sb[d][:],
                                    start=(d == 0), stop=(d == ND - 1))
                            for d in range(ND):
                                nc.tensor.matmul(
                                    out=h3[:], lhsT=w3b[:, d, q * P:(q + 1) * P],
                                    rhs=xT_sb[d][:],
                                    start=(d == 0), stop=(d == ND - 1))
                            sg = ssb.tile([P, TSH], F32, name="sg")
                            nc.scalar.activation(sg[:], h1[:], AF.Silu)
                            nc.vector.tensor_tensor(
                                out=gs_tiles[si][:], in0=sg[:], in1=h3[:], op=OP.mult)

                # ---- shared mm2: z -> out (for this half's token tiles) ----
                with tc.tile_pool(name="sh2_w", bufs=2) as w2p, \
                     tc.tile_pool(name="sh2_sb", bufs=3) as zsb, \
                     tc.tile_pool(name="sh2_ps", bufs=2, space="PSUM") as zps:
                    for ch in range(N_DCH):
                        w2t = w2p.tile([P, NSI, DCH], F32R, name="sw2t", tag="sw2t")
                        for si in range(NSI):
                            nc.sync.dma_start(
                                out=w2t[:, si, :],
                                in_=sw2T[si * P:(si + 1) * P,
                                         ch * DCH:(ch + 1) * DCH].bitcast(F32R))
                        for jj in range(NTH):
                            tj = h * NTH + jj
                            zp = zps.tile([P, DCH], F32, space="PSUM", name="zp")
                            for si in range(NSI):
                                nc.tensor.matmul(
                                    out=zp[:],
                                    lhsT=gs_tiles[si][:, jj * P:(jj + 1) * P],
                                    rhs=w2t[:, si, :],
                                    start=(si == 0), stop=(si == NSI - 1))
                            z_sb = zsb.tile([P, DCH], F32, name="zsb")
                            nc.scalar.copy(z_sb[:], zp[:])
                            nc.sync.dma_start(
                                out=out[tj * P:(tj + 1) * P, ch * DCH:(ch + 1) * DCH],
                                in_=z_sb[:])

    # =================== compaction: build per-expert index lists ==============
    with tc.tile_pool(name="cmp_sb", bufs=1) as csb, \
         tc.tile_pool(name="cmp_ps", bufs=1, space="PSUM") as cps:
        W = NT * E
        # within-tile exclusive prefix (over partitions) per column
        pre_ps = cps.tile([P, W], F32, space="PSUM", name="pre")
        nc.tensor.matmul(out=pre_ps[:], lhsT=ltri_sb[:], rhs=m_all[:],
                         start=True, stop=True)
        # per-(tile,expert) column sums
        cs_ps = cps.tile([1, W], F32, space="PSUM", name="cs")
        nc.tensor.matmul(out=cs_ps[:], lhsT=ones_col[:], rhs=m_all[:],
                         start=True, stop=True)
        cs_sb = csb.tile([1, W], F32)
        nc.scalar.copy(cs_sb[:], cs_ps[:])

        # exclusive cumsum over tiles j (stride E), log-shift trick
        acc = cs_sb
        sh = 1
        while sh < NT:
            pad = csb.tile([1, W + sh * E], F32, name=f"cumpad_{sh}")
            nc.vector.memset(pad[:, :sh * E], 0.0)
            nc.vector.tensor_copy(pad[:, sh * E:], acc[:])
            nxt = csb.tile([1, W], F32, name=f"cum_{sh}")
            nc.vector.tensor_tensor(out=nxt[:], in0=pad[:, sh * E:],
                                    in1=pad[:, :W], op=OP.add)
            acc = nxt
            sh *= 2
        # off = inclusive - colsum + e*C  (exclusive tile offset + expert base)
        off = csb.tile([1, W], F32)
        nc.vector.tensor_tensor(out=off[:], in0=acc[:], in1=cs_sb[:], op=OP.subtract)
        nc.vector.tensor_tensor(out=off[:], in0=off[:], in1=ecols_sb[:, :W], op=OP.add)
        offr = csb.tile([1, W], F32R)
        nc.vector.tensor_copy(offr[:], off[:].bitcast(F32R))
        offb_ps = cps.tile([P, W], F32, space="PSUM", name="offb")
        nc.tensor.matmul(out=offb_ps[:], lhsT=ones_row[:], rhs=offr[:],
                         start=True, stop=True)
        offb = csb.tile([P, W], F32)
        nc.scalar.copy(offb[:], offb_ps[:])

        # dest = prefix + off (+BIG where not routed); scatter token ids
        dest_f = csb.tile([P, W], F32)
        nc.vector.tensor_tensor(out=dest_f[:], in0=pre_ps[:], in1=offb[:], op=OP.add)
        notm = csb.tile([P, W], F32)
        nc.vector.tensor_scalar(notm[:], m_all[:].bitcast(F32), -BIG, BIG,
                                op0=OP.mult, op1=OP.add)
        nc.vector.tensor_tensor(out=dest_f[:], in0=dest_f[:], in1=notm[:], op=OP.add)
        dest_i = csb.tile([P, W], I32)
        nc.vector.tensor_copy(dest_i[:], dest_f[:])

        for j in range(NT):
            vj = csb.tile([P, 8], I32, name=f"vj_{j}")
            nc.vector.tensor_scalar_add(vj[:, :E], iota8_sb[:, :E], j * P)
            for e in range(E):
                nc.gpsimd.indirect_dma_start(
                    out=idx_dram[:],
                    out_offset=bass.IndirectOffsetOnAxis(
                        ap=dest_i[:, j * E + e:j * E + e + 1], axis=0),
                    in_=vj[:, e:e + 1],
                    in_offset=None,
                    bounds_check=EC - 1,
                    oob_is_err=False,
                )

    # =================== routed experts ========================================
    with tc.tile_pool(name="rt_idx", bufs=2) as ixp, \
         tc.tile_pool(name="rt_xg", bufs=3) as xgp, \
         tc.tile_pool(name="rt_xgt", bufs=1) as xtp, \
         tc.tile_pool(name="rt_w", bufs=2) as rwp, \
         tc.tile_pool(name="rt_w2", bufs=2) as rw2p, \
         tc.tile_pool(name="rt_ge", bufs=2) as gep, \
         tc.tile_pool(name="rt_sb", bufs=3) as rsb, \
         tc.tile_pool(name="rt_y", bufs=1) as ryp, \
         tc.tile_pool(name="rt_ps", bufs=2, space="PSUM") as rps, \
         tc.tile_pool(name="rt_tps", bufs=2, space="PSUM") as tps, \
         tc.tile_pool(name="rt_yps", bufs=2, space="PSUM") as yps:
        for e in range(E):
            idxt = []
            sge = []
            for ct in range(NCT):
                it = ixp.tile([P, 1], I32, name=f"idx_{ct}", tag=f"idx_{ct}")
                nc.sync.dma_start(
                    out=it[:],
                    in_=idx_dram[e * C + ct * P:e * C + (ct + 1) * P, :])
                idxt.append(it)
                st = ixp.tile([P, 1], F32, name=f"sg_{ct}", tag=f"sg_{ct}")
                nc.gpsimd.indirect_dma_start(
                    out=st[:], out_offset=None,
                    in_=s_dram[:],
                    in_offset=bass.IndirectOffsetOnAxis(ap=it[:, :1], axis=0),
                    element_offset=e * (TS + 1),
                )
                sge.append(st)

            # gather + transpose x rows -> xgT[:, d, :] = [P(d), C] per d-tile
            xgT = xtp.tile([P, ND, C], F32R, name="xgT")
            for ct in range(NCT):
                xg = xgp.tile([P, D], F32, name="xg")
                nc.gpsimd.indirect_dma_start(
                    out=xg[:], out_offset=None,
                    in_=xs[:],
                    in_offset=bass.IndirectOffsetOnAxis(ap=idxt[ct][:, :1], axis=0),
                )
                for d in range(ND):
                    tp = tps.tile([P, P], F32, space="PSUM", name="tp")
                    nc.tensor.transpose(tp[:], xg[:, d * P:(d + 1) * P], identity[:])
                    nc.vector.tensor_copy(
                        out=xgT[:, d, ct * P:(ct + 1) * P], in_=tp[:].bitcast(F32R))

            # mm1: ge = silu(w1 xg) * (w3 xg), [P(i), C] per i-tile
            ge = gep.tile([P, NI, C], F32R, name="ge")
            for i in range(NI):
                w1b = rwp.tile([P, ND, P], F32R, name="w1b", tag="w1b")
                w3b = rwp.tile([P, ND, P], F32R, name="w3b", tag="w3b")
                for d in range(ND):
                    nc.sync.dma_start(
                        out=w1b[:, d, :],
                        in_=w1T[e, d * P:(d + 1) * P, i * P:(i + 1) * P].bitcast(F32R))
                    nc.sync.dma_start(
                        out=w3b[:, d, :],
                        in_=w3T[e, d * P:(d + 1) * P, i * P:(i + 1) * P].bitcast(F32R))
                h1 = rps.tile([P, C], F32, space="PSUM", name="h1r")
                h3 = rps.tile([P, C], F32, space="PSUM", name="h3r")
                for d in range(ND):
                    nc.tensor.matmul(
                        out=h1[:], lhsT=w1b[:, d, :], rhs=xgT[:, d, :],
                        start=(d == 0), stop=(d == ND - 1))
                for d in range(ND):
                    nc.tensor.matmul(
                        out=h3[:], lhsT=w3b[:, d, :], rhs=xgT[:, d, :],
                        start=(d == 0), stop=(d == ND - 1))
                sg = rsb.tile([P, C], F32, name="sgr")
                nc.scalar.activation(sg[:], h1[:], AF.Silu)
                nc.vector.tensor_tensor(out=ge[:, i, :], in0=sg[:], in1=h3[:],
                                        op=OP.mult)

            # mm2: y = ge @ w2, scaled by routing weight, scatter-add to out
            y_sb = [ryp.tile([P, D], F32, name=f"ysb_{ct}", tag=f"ysb_{ct}")
                    for ct in range(NCT)]
            for ch in range(N_DCH):
                w2t = rw2p.tile([P, NI, DCH], F32R, name="w2t", tag="w2t")
                for i in range(NI):
                    nc.sync.dma_start(
                        out=w2t[:, i, :],
                        in_=w2T[e, i * P:(i + 1) * P,
                                ch * DCH:(ch + 1) * DCH].bitcast(F32R))
                for ct in range(NCT):
                    yp = yps.tile([P, DCH], F32, space="PSUM", name="yp")
                    for i in range(NI):
                        nc.tensor.matmul(
                            out=yp[:], lhsT=ge[:, i, ct * P:(ct + 1) * P],
                            rhs=w2t[:, i, :], start=(i == 0), stop=(i == NI - 1))
                    nc.scalar.mul(y_sb[ct][:, ch * DCH:(ch + 1) * DCH], yp[:],
                                  sge[ct][:, :1])
            for ct in range(NCT):
                nc.gpsimd.indirect_dma_start(
                    out=out[:],
                    out_offset=bass.IndirectOffsetOnAxis(ap=idxt[ct][:, :1], axis=0),
                    in_=y_sb[ct][:],
                    in_offset=None,
                    bounds_check=TS - 1,
                    oob_is_err=False,
                    compute_op=OP.add,
                )


def _declare_io(nc, dims, debug_internals=False):
    TS, D, E, I, SI, C = (dims[k] for k in ("TS", "D", "E", "I", "SI", "C"))
    NT = TS // P
    ikind = "ExternalOutput" if debug_internals else "Internal"
    io = {}
    io["xs"] = nc.dram_tensor("xs", [TS + 1, D], F32, kind="ExternalInput").ap()
    io["xT"] = nc.dram_tensor("xT", [D, TS], F32, kind="ExternalInput").ap()
    io["gwT"] = nc.dram_tensor("gwT", [D, E], F32, kind="ExternalInput").ap()
    io["w1T"] = nc.dram_tensor("w1T", [E, D, I], F32, kind="ExternalInput").ap()
    io["w3T"] = nc.dram_tensor("w3T", [E, D, I], F32, kind="ExternalInput").ap()
    io["w2T"] = nc.dram_tensor("w2T", [E, I, D], F32, kind="ExternalInput").ap()
    io["sw1T"] = nc.dram_tensor("sw1T", [D, SI], F32, kind="ExternalInput").ap()
    io["sw3T"] = nc.dram_tensor("sw3T", [D, SI], F32, kind="ExternalInput").ap()
    io["sw2T"] = nc.dram_tensor("sw2T", [SI, D], F32, kind="ExternalInput").ap()
    io["ltri"] = nc.dram_tensor("ltri", [P, P], F32, kind="ExternalInput").ap()
    io["iota8"] = nc.dram_tensor("iota8", [P, 8], I32, kind="ExternalInput").ap()
    io["ecols"] = nc.dram_tensor("ecols", [1, NT * E], F32, kind="ExternalInput").ap()
    io["out"] = nc.dram_tensor("out", [TS, D], F32, kind="ExternalOutput").ap()
    io["idx_dram"] = nc.dram_tensor("idx_dram", [E * C, 1], I32, kind=ikind).ap()
    io["s_dram"] = nc.dram_tensor("s_dram", [E * (TS + 1), 1], F32, kind=ikind).ap()
    return io


@lru_cache(maxsize=2)
def _build(dims_key, debug_internals=False):
    dims = dict(dims_key)
    nc = bacc.Bacc("TRN2", target_bir_lowering=False, debug=False,
                   num_devices=N_CORES)
    io = _declare_io(nc, dims, debug_internals=debug_internals)
    with tile.TileContext(nc) as tc:
        with ExitStack() as ctx:
            build_moe(nc, tc, ctx, io, dims)
    nc.compile()
    return nc


def host_consts(dims):
    TS, E, C = dims["TS"], dims["E"], dims["C"]
    NT = TS // P
    # lhsT[k=p', m=p] = 1 iff p' < p  (strictly-lower-triangular, transposed)
    ltri = np.tril(np.ones((P, P), np.float32), -1).T.copy()
    iota8 = np.tile(np.arange(P, dtype=np.int32)[:, None], (1, 8))
    ecols = np.zeros((1, NT * E), np.float32)
    for j in range(NT):
        for e in range(E):
            ecols[0, j * E + e] = e * C
    return ltri, iota8, ecols


def make_in_maps(x, gate_w, w1, w2, w3, sw1, sw2, sw3, dims, n_cores=N_CORES):
    TS, D = dims["TS"], dims["D"]
    T = TS * n_cores
    xt = np.ascontiguousarray(x.reshape(T, D).astype(np.float32, copy=False))
    xT_full = np.ascontiguousarray(xt.T)
    shared = dict(
        gwT=np.ascontiguousarray(gate_w.T),
        w1T=np.ascontiguousarray(w1.transpose(0, 2, 1)),
        w3T=np.ascontiguousarray(w3.transpose(0, 2, 1)),
        w2T=np.ascontiguousarray(w2.transpose(0, 2, 1)),
        sw1T=np.ascontiguousarray(sw1.T),
        sw3T=np.ascontiguousarray(sw3.T),
        sw2T=np.ascontiguousarray(sw2.T),
    )
    ltri, iota8, ecols = host_consts(dims)
    shared.update(ltri=ltri, iota8=iota8, ecols=ecols)
    in_maps = []
    for c in range(n_cores):
        xs = np.zeros((TS + 1, D), np.float32)
        xs[:TS] = xt[c * TS:(c + 1) * TS]
        xTs = np.ascontiguousarray(xT_full[:, c * TS:(c + 1) * TS])
        in_maps.append(dict(xs=xs, xT=xTs, **shared))
    return in_maps


def kernel(x, gate_w, w1, w2, w3, sw1, sw2, sw3):
    dims = dict(FULL)
    B, S, D = x.shape
    nc = _build(tuple(sorted(dims.items())))
    in_maps = make_in_maps(x, gate_w, w1, w2, w3, sw1, sw2, sw3, dims)
    res = run_bass_kernel_spmd(nc, in_maps, core_ids=list(range(N_CORES)))
    outs = [res.results[c]["out"] for c in range(N_CORES)]
    y = np.concatenate(outs, axis=0).reshape(B, S, D)
    return y


# revision 20
# speedup vs baseline: 2.5177x; 2.5177x over previous
"""MoE (top-2 of 8 experts + shared SwiGLU) Trainium2 kernel.

Strategy: data-parallel over tokens across 8 NeuronCores (1024 tokens each).
Each core runs an identical program:
  - shared-expert SwiGLU mm1 over the slice (fp16 matmuls, fp32 accumulate)
  - gate softmax + top-2 on its token slice (TRUE fp32 matmuls: top-2
    selection must match the fp32 reference's ordering exactly)
  - on-device compaction, matmul-only: a triangular-matmul prefix sum ranks
    each routed token; an is_equal one-hot against an iota row and one
    matmul per (expert, chunk) gathers the token ids AND routing weights
    into SBUF index tiles (no indirect DMA, no DRAM round-trip)
  - shared mm2 writes z into the output
  - per expert: indirect gather of x rows -> PE transpose -> SwiGLU (fp16)
    -> scale by routing weight -> indirect scatter-ADD into the output slice
Output per core is its own [1024, 2048] slice; the host just concatenates.

Weight layouts are chosen so every weight DMA moves >=0.75KB contiguous
per partition and one DMA covers many tiles (reshaped-AP batching).
"""

import math
from contextlib import ExitStack
from functools import lru_cache

import numpy as np

import concourse.bass as bass
import concourse.mybir as mybir
import concourse.tile as tile
from concourse import bacc
from concourse.bass_utils import run_bass_kernel_spmd
from concourse.masks import make_identity

F32 = mybir.dt.float32
F32R = mybir.dt.float32r
F16 = mybir.dt.float16
I32 = mybir.dt.int32
AF = mybir.ActivationFunctionType
OP = mybir.AluOpType

P = 128

# Full-problem dims (graded input is B=4,S=2048,D=2048,E=8,I=1408,SI=2816)
FULL = dict(TS=1024, D=2048, E=8, I=1408, SI=2816, C=384)
N_CORES = 8
BIG = 1.0e9  # sentinel rank for unrouted tokens (never matches the iota row)
IGRP = 4     # inter-dim tiles per batched weight DMA


def build_moe(nc, tc, ctx, io, dims):
    """Emit the tile program. io: dict of DRAM APs. dims: dict of sizes."""
    TS, D, E, I, SI, C = (dims[k] for k in ("TS", "D", "E", "I", "SI", "C"))
    NT = TS // P          # token tiles in slice
    ND = D // P           # d (model dim) tiles
    NI = I // P           # routed inter-dim tiles
    NSI = SI // P         # shared inter-dim tiles
    NCT = C // P          # capacity tiles per expert
    DCH = min(512, D)     # moving chunk over d (mm2 outputs)
    N_DCH = D // DCH
    TCH = min(512, TS)    # moving chunk over tokens (shared mm1)
    N_TCH = TS // TCH
    W = NT * E

    xs, xT, xT16 = io["xs"], io["xT"], io["xT16"]
    gwT = io["gwT"]
    w1L, w3L, w2L = io["w1L"], io["w3L"], io["w2L"]
    sw1L, sw3L, sw2L = io["sw1L"], io["sw3L"], io["sw2L"]
    ltri, iota8, iotab = io["ltri"], io["iota8"], io["iotab"]
    out = io["out"]

    const_pool = ctx.enter_context(tc.tile_pool(name="const", bufs=1))

    identity = const_pool.tile([P, P], F16)
    make_identity(nc, identity[:])
    ltri_sb = const_pool.tile([P, P], F32R)
    nc.sync.dma_start(out=ltri_sb[:], in_=ltri[:].bitcast(F32R))
    iota8_sb = const_pool.tile([P, 8], I32)
    nc.sync.dma_start(out=iota8_sb[:], in_=iota8[:])
    iotab_sb = const_pool.tile([P, C], F32)
    nc.sync.dma_start(out=iotab_sb[:], in_=iotab[:])
    if32 = const_pool.tile([P, 1], F32)
    nc.vector.tensor_copy(if32[:], iota8_sb[:, :1])
    ones_f = const_pool.tile([P, 1], F32)
    nc.vector.memset(ones_f[:], 1.0)
    ones_col = const_pool.tile([P, 1], F32R)
    nc.vector.tensor_copy(ones_col[:], ones_f[:].bitcast(F32R))
    ones_rf = const_pool.tile([1, P], F32)
    nc.vector.memset(ones_rf[:], 1.0)
    ones_row = const_pool.tile([1, P], F32R)
    nc.vector.tensor_copy(ones_row[:], ones_rf[:].bitcast(F32R))
    # gate weights in TRUE fp32 (exact top-2 selection)
    gwT_sb = []
    for d in range(ND):
        t = const_pool.tile([P, E], F32, name=f"gwT_{d}", tag=f"gwT_{d}")
        nc.sync.dma_start(out=t[:], in_=gwT[d * P:(d + 1) * P, :])
        gwT_sb.append(t)

    rt_pool = ctx.enter_context(tc.tile_pool(name="routing", bufs=1))
    m_all = rt_pool.tile([P, W], F32R)   # top-2 masks, col = j*E + e
    s_all = rt_pool.tile([P, W], F32)    # routing weights, col = j*E + e
    pm_all = rt_pool.tile([P, W], F32)   # per-token rank in expert list (or BIG)
    rhs_j = [rt_pool.tile([P, 2 + E], F32, name=f"rhs_{j}", tag=f"rhs_{j}")
             for j in range(NT)]
    # per-(expert, chunk) token-index + routing-weight tiles
    idx_pool = ctx.enter_context(tc.tile_pool(name="idxp", bufs=1))
    idxt = [[idx_pool.tile([P, 1], I32, name=f"idx_{e}_{ct}", tag=f"idx_{e}_{ct}")
             for ct in range(NCT)] for e in range(E)]
    sget = [[idx_pool.tile([P, 1], F32, name=f"sg_{e}_{ct}", tag=f"sg_{e}_{ct}")
             for ct in range(NCT)] for e in range(E)]

    # =================== Phase 2: shared mm1 (gS = silu(sw1 x)*(sw3 x)) ========
    gs_tiles = []
    with tc.tile_pool(name="gs", bufs=1) as gs_pool:
        with tc.tile_pool(name="xt16", bufs=1) as xt16p:
            xT_sb = []
            for d in range(ND):
                t = xt16p.tile([P, TS], F16, name=f"xT16_{d}", tag=f"xT16_{d}")
                nc.sync.dma_start(out=t[:], in_=xT16[d * P:(d + 1) * P, :])
                xT_sb.append(t)
            for si in range(NSI):
                gs_tiles.append(
                    gs_pool.tile([P, TS], F16, name=f"gs_{si}", tag=f"gs_{si}"))
            n_grp = math.ceil(NSI / IGRP)
            with tc.tile_pool(name="sh1_w", bufs=2) as swp, \
                 tc.tile_pool(name="sh1_sb", bufs=3) as ssb, \
                 tc.tile_pool(name="sh1_ps", bufs=2, space="PSUM") as sps:
                for g in range(n_grp):
                    si0 = g * IGRP
                    ng = min(IGRP, NSI - si0)
                    w1b = swp.tile([P, ND, IGRP * P], F16, name="sw1b", tag="sw1b")
                    w3b = swp.tile([P, ND, IGRP * P], F16, name="sw3b", tag="sw3b")
                    nc.sync.dma_start(
                        out=w1b[:, :, :ng * P],
                        in_=sw1L[:].rearrange("dt p i -> p dt i")[
                            :, :, si0 * P:(si0 + ng) * P])
                    nc.sync.dma_start(
                        out=w3b[:, :, :ng * P],
                        in_=sw3L[:].rearrange("dt p i -> p dt i")[
                            :, :, si0 * P:(si0 + ng) * P])
                    for q in range(ng):
                        si = si0 + q
                        for hc in range(N_TCH):
                            h1 = sps.tile([P, TCH], F32, space="PSUM", name="h1")
                            h3 = sps.tile([P, TCH], F32, space="PSUM", name="h3")
                            for d in range(ND):
                                nc.tensor.matmul(
                                    out=h1[:], lhsT=w1b[:, d, q * P:(q + 1) * P],
                                    rhs=xT_sb[d][:, hc * TCH:(hc + 1) * TCH],
                                    start=(d == 0), stop=(d == ND - 1))
                            for d in range(ND):
                                nc.tensor.matmul(
                                    out=h3[:], lhsT=w3b[:, d, q * P:(q + 1) * P],
                                    rhs=xT_sb[d][:, hc * TCH:(hc + 1) * TCH],
                                    start=(d == 0), stop=(d == ND - 1))
                            sg = ssb.tile([P, TCH], F32, name="sg")
                            nc.scalar.activation(sg[:], h1[:], AF.Silu)
                            nc.vector.tensor_tensor(
                                out=gs_tiles[si][:, hc * TCH:(hc + 1) * TCH],
                                in0=sg[:], in1=h3[:], op=OP.mult)

        # =================== Phase 1: gate + routing ===========================
        with tc.tile_pool(name="gate_sb", bufs=2) as gsb, \
             tc.tile_pool(name="gate_x", bufs=1) as gxp, \
             tc.tile_pool(name="gate_ps", bufs=2, space="PSUM") as gps:
            xf_sb = []
            for d in range(ND):
                t = gxp.tile([P, TS], F32, name=f"xf_{d}", tag=f"xf_{d}")
                nc.sync.dma_start(out=t[:], in_=xT[d * P:(d + 1) * P, :])
                xf_sb.append(t)
            for j in range(NT):
                sc_ps = gps.tile([P, E], F32, space="PSUM", name="sc")
                for d in range(ND):
                    nc.tensor.matmul(
                        out=sc_ps[:],
                        lhsT=xf_sb[d][:, j * P:(j + 1) * P],
                        rhs=gwT_sb[d][:],
                        start=(d == 0), stop=(d == ND - 1),
                    )
                es = gsb.tile([P, E], F32, name="es")
                nc.scalar.activation(es[:], sc_ps[:], AF.Exp)
                zsum = gsb.tile([P, 1], F32, name="zsum")
                nc.vector.tensor_reduce(zsum[:], es[:], axis=mybir.AxisListType.X,
                                        op=OP.add)
                rec = gsb.tile([P, 1], F32, name="rec")
                nc.vector.reciprocal(rec[:], zsum[:])
                prob = gsb.tile([P, E], F32, name="prob")
                nc.vector.tensor_scalar_mul(prob[:], es[:], rec[:, :1])
                top8 = gsb.tile([P, 8], F32, name="top8")
                nc.vector.max(out=top8[:], in_=prob[:])
                # mask = prob >= second_max  (top-2)
                nc.vector.tensor_tensor(
                    out=m_all[:, j * E:(j + 1) * E],
                    in0=prob[:], in1=top8[:, 1:2].to_broadcast([P, E]),
                    op=OP.is_ge,
                )
                # routing weight s = prob * mask
                nc.vector.tensor_tensor(
                    out=s_all[:, j * E:(j + 1) * E], in0=prob[:],
                    in1=m_all[:, j * E:(j + 1) * E].bitcast(F32), op=OP.mult)
                # rhs for the compaction gather-matmul: [token_id | s row]
                nc.vector.tensor_scalar_add(rhs_j[j][:, 0:1], if32[:], float(j * P))
                nc.vector.tensor_copy(rhs_j[j][:, 1:1 + E],
                                      s_all[:, j * E:(j + 1) * E])
                nc.vector.memset(rhs_j[j][:, 1 + E:2 + E], 1.0)

        # ====== compaction part A: rank every routed token within its expert ===
        with tc.tile_pool(name="cmp_sb", bufs=1) as csb, \
             tc.tile_pool(name="cmp_ps", bufs=1, space="PSUM") as cps:
            # within-tile exclusive prefix (over partitions) per column
            pre_ps = cps.tile([P, W], F32, space="PSUM", name="pre")
            nc.tensor.matmul(out=pre_ps[:], lhsT=ltri_sb[:], rhs=m_all[:],
                             start=True, stop=True)
            # per-(tile,expert) column sums
            cs_ps = cps.tile([1, W], F32, space="PSUM", name="cs")
            nc.tensor.matmul(out=cs_ps[:], lhsT=ones_col[:], rhs=m_all[:],
                             start=True, stop=True)
            cs_sb = csb.tile([1, W], F32)
            nc.scalar.copy(cs_sb[:], cs_ps[:])

            # exclusive cumsum over tiles j (stride E), log-shift trick
            acc = cs_sb
            sh = 1
            while sh < NT:
                pad = csb.tile([1, W + sh * E], F32, name=f"cumpad_{sh}")
                nc.vector.memset(pad[:, :sh * E], 0.0)
                nc.vector.tensor_copy(pad[:, sh * E:], acc[:])
                nxt = csb.tile([1, W], F32, name=f"cum_{sh}")
                nc.vector.tensor_tensor(out=nxt[:], in0=pad[:, sh * E:],
                                        in1=pad[:, :W], op=OP.add)
                acc = nxt
                sh *= 2
            off = csb.tile([1, W], F32)
            nc.vector.tensor_tensor(out=off[:], in0=acc[:], in1=cs_sb[:],
                                    op=OP.subtract)
            offr = csb.tile([1, W], F32R)
            nc.vector.tensor_copy(offr[:], off[:].bitcast(F32R))
            offb_ps = cps.tile([P, W], F32, space="PSUM", name="offb")
            nc.tensor.matmul(out=offb_ps[:], lhsT=ones_row[:], rhs=offr[:],
                             start=True, stop=True)
            offb = csb.tile([P, W], F32)
            nc.scalar.copy(offb[:], offb_ps[:])

            # rank = prefix + tile offset; +BIG where not routed
            nc.vector.tensor_tensor(out=pm_all[:], in0=pre_ps[:], in1=offb[:],
                                    op=OP.add)
            notm = csb.tile([P, W], F32)
            nc.vector.tensor_scalar(notm[:], m_all[:].bitcast(F32), -BIG, BIG,
                                    op0=OP.mult, op1=OP.add)
            nc.vector.tensor_tensor(out=pm_all[:], in0=pm_all[:], in1=notm[:],
                                    op=OP.add)

        # =================== Phase 3: shared mm2, z -> out =====================
        with tc.tile_pool(name="sh2_w", bufs=2) as w2p, \
             tc.tile_pool(name="sh2_sb", bufs=3) as zsb, \
             tc.tile_pool(name="sh2_ps", bufs=2, space="PSUM") as zps:
            for ch in range(N_DCH):
                w2t = w2p.tile([P, NSI, DCH], F16, name="sw2t", tag="sw2t")
                nc.sync.dma_start(
                    out=w2t[:],
                    in_=sw2L[:].rearrange("si p d -> p si d")[
                        :, :, ch * DCH:(ch + 1) * DCH])
                for tj in range(NT):
                    zp = zps.tile([P, DCH], F32, space="PSUM", name="zp")
                    for si in range(NSI):
                        nc.tensor.matmul(
                            out=zp[:],
                            lhsT=gs_tiles[si][:, tj * P:(tj + 1) * P],
                            rhs=w2t[:, si, :],
                            start=(si == 0), stop=(si == NSI - 1))
                    z_sb = zsb.tile([P, DCH], F32, name="zsb")
                    nc.scalar.copy(z_sb[:], zp[:])
                    nc.sync.dma_start(
                        out=out[tj * P:(tj + 1) * P, ch * DCH:(ch + 1) * DCH],
                        in_=z_sb[:])

    # ====== compaction part B: gather token ids + weights per (expert, chunk) ==
    # one-hot(eq) x [token_id | s] matmul; unmatched ranks (pads) give 0s.
    with tc.tile_pool(name="eq_sb", bufs=2 * NT) as esb, \
         tc.tile_pool(name="eq_ps", bufs=2, space="PSUM") as eps:
        for e in range(E):
            eqs = []
            for j in range(NT):
                eq = esb.tile([P, C], F32, name=f"eq_{j}", tag=f"eq_{j}")
                nc.vector.tensor_tensor(
                    out=eq[:],
                    in0=pm_all[:, j * E + e:j * E + e + 1].to_broadcast([P, C]),
                    in1=iotab_sb[:], op=OP.is_equal)
                eqs.append(eq)
            for ct in range(NCT):
                gp = eps.tile([P, 2 + E], F32, space="PSUM", name="gp")
                for j in range(NT):
                    nc.tensor.matmul(
                        out=gp[:], lhsT=eqs[j][:, ct * P:(ct + 1) * P],
                        rhs=rhs_j[j][:], start=(j == 0), stop=(j == NT - 1))
                padv = esb.tile([P, 1], F32, name="padv")
                nc.vector.tensor_scalar(padv[:], gp[:, 1 + E:2 + E],
                                        float(-TS), float(TS),
                                        op0=OP.mult, op1=OP.add)
                idx_f = esb.tile([P, 1], F32, name="idx_f")
                nc.vector.tensor_tensor(out=idx_f[:], in0=gp[:, 0:1],
                                        in1=padv[:], op=OP.add)
                nc.vector.tensor_copy(idxt[e][ct][:], idx_f[:])
                nc.vector.tensor_copy(sget[e][ct][:], gp[:, 1 + e:2 + e])
                if "idx_dbg" in io:
                    nc.sync.dma_start(
                        out=io["idx_dbg"][e * C + ct * P:e * C + (ct + 1) * P, :],
                        in_=idxt[e][ct][:])
                    nc.sync.dma_start(
                        out=io["s_dbg"][e * C + ct * P:e * C + (ct + 1) * P, :],
                        in_=sget[e][ct][:])

    # =================== routed experts ========================================
    n_igrp = math.ceil(NI / IGRP)
    with tc.tile_pool(name="rt_xg", bufs=3) as xgp, \
         tc.tile_pool(name="rt_xgt", bufs=2) as xtp, \
         tc.tile_pool(name="rt_w", bufs=2) as rwp, \
         tc.tile_pool(name="rt_w2", bufs=2) as rw2p, \
         tc.tile_pool(name="rt_ge", bufs=2) as gep, \
         tc.tile_pool(name="rt_sb", bufs=3) as rsb, \
         tc.tile_pool(name="rt_y", bufs=1) as ryp, \
         tc.tile_pool(name="rt_ps", bufs=2, space="PSUM") as rps, \
         tc.tile_pool(name="rt_tps", bufs=2, space="PSUM") as tps, \
         tc.tile_pool(name="rt_yps", bufs=2, space="PSUM") as yps:
        for e in range(E):
            # gather + transpose x rows -> xgT[:, d, :] = [P(d), C] per d-tile
            xgT = xtp.tile([P, ND, C], F16, name="xgT")
            for ct in range(NCT):
                xg = xgp.tile([P, D], F16, name="xg")
                nc.gpsimd.indirect_dma_start(
                    out=xg[:], out_offset=None,
                    in_=xs[:],
                    in_offset=bass.IndirectOffsetOnAxis(ap=idxt[e][ct][:, :1],
                                                        axis=0),
                )
                for d in range(ND):
                    tp = tps.tile([P, P], F16, space="PSUM", name="tp")
                    nc.tensor.transpose(tp[:], xg[:, d * P:(d + 1) * P],
                                        identity[:])
                    nc.vector.tensor_copy(
                        out=xgT[:, d, ct * P:(ct + 1) * P], in_=tp[:])

            # mm1: ge = silu(w1 xg) * (w3 xg), [P(i), C] per i-tile
            ge = gep.tile([P, NI, C], F16, name="ge")
            for g in range(n_igrp):
                i0 = g * IGRP
                ng = min(IGRP, NI - i0)
                w1b = rwp.tile([P, ND, IGRP * P], F16, name="w1b", tag="w1b")
                w3b = rwp.tile([P, ND, IGRP * P], F16, name="w3b", tag="w3b")
                nc.sync.dma_start(
                    out=w1b[:, :, :ng * P],
                    in_=w1L[e].rearrange("dt p i -> p dt i")[
                        :, :, i0 * P:(i0 + ng) * P])
                nc.sync.dma_start(
                    out=w3b[:, :, :ng * P],
                    in_=w3L[e].rearrange("dt p i -> p dt i")[
                        :, :, i0 * P:(i0 + ng) * P])
                for q in range(ng):
                    i = i0 + q
                    h1 = rps.tile([P, C], F32, space="PSUM", name="h1r")
                    h3 = rps.tile([P, C], F32, space="PSUM", name="h3r")
                    for d in range(ND):
                        nc.tensor.matmul(
                            out=h1[:], lhsT=w1b[:, d, q * P:(q + 1) * P],
                            rhs=xgT[:, d, :], start=(d == 0), stop=(d == ND - 1))
                    for d in range(ND):
                        nc.tensor.matmul(
                            out=h3[:], lhsT=w3b[:, d, q * P:(q + 1) * P],
                            rhs=xgT[:, d, :], start=(d == 0), stop=(d == ND - 1))
                    sg = rsb.tile([P, C], F32, name="sgr")
                    nc.scalar.activation(sg[:], h1[:], AF.Silu)
                    nc.vector.tensor_tensor(out=ge[:, i, :], in0=sg[:], in1=h3[:],
                                            op=OP.mult)

            # mm2: y = ge @ w2, scaled by routing weight, scatter-add to out
            y_sb = [ryp.tile([P, D], F32, name=f"ysb_{ct}", tag=f"ysb_{ct}")
                    for ct in range(NCT)]
            for ch in range(N_DCH):
                w2t = rw2p.tile([P, NI, DCH], F16, name="w2t", tag="w2t")
                nc.sync.dma_start(
                    out=w2t[:],
                    in_=w2L[e].rearrange("i p d -> p i d")[
                        :, :, ch * DCH:(ch + 1) * DCH])
                for ct in range(NCT):
                    yp = yps.tile([P, DCH], F32, space="PSUM", name="yp")
                    for i in range(NI):
                        nc.tensor.matmul(
                            out=yp[:], lhsT=ge[:, i, ct * P:(ct + 1) * P],
                            rhs=w2t[:, i, :], start=(i == 0), stop=(i == NI - 1))
                    nc.scalar.mul(y_sb[ct][:, ch * DCH:(ch + 1) * DCH], yp[:],
                                  sget[e][ct][:, :1])
            for ct in range(NCT):
                nc.gpsimd.indirect_dma_start(
                    out=out[:],
                    out_offset=bass.IndirectOffsetOnAxis(ap=idxt[e][ct][:, :1],
                                                        axis=0),
                    in_=y_sb[ct][:],
                    in_offset=None,
                    bounds_check=TS - 1,
                    oob_is_err=False,
                    compute_op=OP.add,
                )


def _declare_io(nc, dims, debug_internals=False):
    TS, D, E, I, SI, C = (dims[k] for k in ("TS", "D", "E", "I", "SI", "C"))
    ND, NI, NSI = D // P, I // P, SI // P
    io = {}
    io["xs"] = nc.dram_tensor("xs", [TS + 1, D], F16, kind="ExternalInput").ap()
    io["xT"] = nc.dram_tensor("xT", [D, TS], F32, kind="ExternalInput").ap()
    io["xT16"] = nc.dram_tensor("xT16", [D, TS], F16, kind="ExternalInput").ap()
    io["gwT"] = nc.dram_tensor("gwT", [D, E], F32, kind="ExternalInput").ap()
    io["w1L"] = nc.dram_tensor("w1L", [E, ND, P, I], F16, kind="ExternalInput").ap()
    io["w3L"] = nc.dram_tensor("w3L", [E, ND, P, I], F16, kind="ExternalInput").ap()
    io["w2L"] = nc.dram_tensor("w2L", [E, NI, P, D], F16, kind="ExternalInput").ap()
    io["sw1L"] = nc.dram_tensor("sw1L", [ND, P, SI], F16, kind="ExternalInput").ap()
    io["sw3L"] = nc.dram_tensor("sw3L", [ND, P, SI], F16, kind="ExternalInput").ap()
    io["sw2L"] = nc.dram_tensor("sw2L", [NSI, P, D], F16, kind="ExternalInput").ap()
    io["ltri"] = nc.dram_tensor("ltri", [P, P], F32, kind="ExternalInput").ap()
    io["iota8"] = nc.dram_tensor("iota8", [P, 8], I32, kind="ExternalInput").ap()
    io["iotab"] = nc.dram_tensor("iotab", [P, C], F32, kind="ExternalInput").ap()
    io["out"] = nc.dram_tensor("out", [TS, D], F32, kind="ExternalOutput").ap()
    if debug_internals:
        io["idx_dbg"] = nc.dram_tensor("idx_dbg", [E * C, 1], I32,
                                       kind="ExternalOutput").ap()
        io["s_dbg"] = nc.dram_tensor("s_dbg", [E * C, 1], F32,
                                     kind="ExternalOutput").ap()
    return io


@lru_cache(maxsize=2)
def _build(dims_key, debug_internals=False):
    dims = dict(dims_key)
    nc = bacc.Bacc("TRN2", target_bir_lowering=False, debug=False,
                   num_devices=N_CORES)
    io = _declare_io(nc, dims, debug_internals=debug_internals)
    with tile.TileContext(nc) as tc:
        with ExitStack() as ctx:
            build_moe(nc, tc, ctx, io, dims)
    nc.compile()
    return nc


def host_consts(dims):
    C = dims["C"]
    # lhsT[k=p', m=p] = 1 iff p' < p  (strictly-lower-triangular, transposed)
    ltri = np.tril(np.ones((P, P), np.float32), -1).T.copy()
    iota8 = np.tile(np.arange(P, dtype=np.int32)[:, None], (1, 8))
    iotab = np.tile(np.arange(C, dtype=np.float32)[None, :], (P, 1))
    return ltri, iota8, iotab


def make_in_maps(x, gate_w, w1, w2, w3, sw1, sw2, sw3, dims, n_cores=N_CORES):
    TS, D, E, I, SI = (dims[k] for k in ("TS", "D", "E", "I", "SI"))
    ND, NI, NSI = D // P, I // P, SI // P
    T = TS * n_cores
    xt = np.ascontiguousarray(x.reshape(T, D).astype(np.float32, copy=False))
    xT_full = np.ascontiguousarray(xt.T)
    xT16_full = xT_full.astype(np.float16)
    f16 = lambda a: np.ascontiguousarray(a).astype(np.float16)
    shared = dict(
        gwT=np.ascontiguousarray(gate_w.T),
        w1L=f16(w1.transpose(0, 2, 1)).reshape(E, ND, P, I),
        w3L=f16(w3.transpose(0, 2, 1)).reshape(E, ND, P, I),
        w2L=f16(w2.transpose(0, 2, 1)).reshape(E, NI, P, D),
        sw1L=f16(sw1.T).reshape(ND, P, SI),
        sw3L=f16(sw3.T).reshape(ND, P, SI),
        sw2L=f16(sw2.T).reshape(NSI, P, D),
    )
    ltri, iota8, iotab = host_consts(dims)
    shared.update(ltri=ltri, iota8=iota8, iotab=iotab)
    in_maps = []
    for c in range(n_cores):
        xs = np.zeros((TS + 1, D), np.float16)
        xs[:TS] = xt[c * TS:(c + 1) * TS].astype(np.float16)
        xTs = np.ascontiguousarray(xT_full[:, c * TS:(c + 1) * TS])
        xTs16 = np.ascontiguousarray(xT16_full[:, c * TS:(c + 1) * TS])
        in_maps.append(dict(xs=xs, xT=xTs, xT16=xTs16, **shared))
    return in_maps


def kernel(x, gate_w, w1, w2, w3, sw1, sw2, sw3):
    dims = dict(FULL)
    B, S, D = x.shape
    nc = _build(tuple(sorted(dims.items())))
    in_maps = make_in_maps(x, gate_w, w1, w2, w3, sw1, sw2, sw3, dims)
    res = run_bass_kernel_spmd(nc, in_maps, core_ids=list(range(N_CORES)))
    outs = [res.results[c]["out"] for c in range(N_CORES)]
    y = np.concatenate(outs, axis=0).reshape(B, S, D)
    return y
